# revision 1
# baseline (speedup 1.0000x reference)
"""KAARMA-style multi-cell kernel recurrence on 8 Trainium2 NeuronCores.

Math reformulation (validated vs reference to ~1e-6 rel):
  per step t, per batch b, for every dictionary atom (c, m) [cm = 800 atoms]:
    phi[b,cm]  = exp(-|s_b - S_cm|^2 - (x_tb - U_cm)^2)
    gate[b,c]  = softmax_c(MLP(x_tb))               (precomputable, x-only)
    s'_b       = sum_cm gate[b,cell(cm)] * phi[b,cm] * A[cm,:]
  Expand the squares and fold the gate into the exponent:
    psi[b,cm]  = exp( 2 s_b.S_cm - |s_b|^2 + 2 x U_cm - x^2 + logsoftmax_c )
    s'_b       = sum_cm psi[b,cm] * (A[cm,:] * exp(-|S_cm|^2 - U_cm^2))
  so one step = matmul([K,B] -> [800,B]) -> exp -> matmul([800,B] -> [16,B]).

Contraction-row layout (engine SBUF writes must start at partition 0/32/64/96,
so segments are 32-aligned and the gaps carry zero weights):
  rows  0:16  s            (weights 2*S)
  rows 32:48  s^2          (weights -1)
  row  64     x_t          (weights 2*U)
  rows 65:73  one-hot cell (data = lg[b,c] - x_t^2)
K = 73; gap rows 16:32, 48:64 are zeroed once and have zero weight columns.

Sharding: pure data parallel, batch 512 -> 64 per core on 8 cores.
Device layout is feature-major: state sT [16, B_local] in PSUM, psi chunks
[128, B_local] (7 chunks of 128 atoms, cm padded 800 -> 896).
"""

import numpy as np

N_CORES = 8
CM = 800
CM_PAD = 896
NCHUNK = 7
KROWS = 73
NSTATE = 16
STAGE = 64  # recurrence steps per staging DMA

_PROGRAM_CACHE = {}


def _build_program(B_local, T, rep=1):
    import concourse.bass as bass
    import concourse.bacc as bacc
    import concourse.tile as tile
    from concourse import mybir
    from contextlib import ExitStack

    f32 = mybir.dt.float32
    Act = mybir.ActivationFunctionType

    # Bacc (not Bass): its compile() runs generate_event_semaphores, which
    # splits multi-wait instructions (TRN2 allows 1 wait per instruction)
    nc = bacc.Bacc("TRN2", target_bir_lowering=False, debug=False)
    W_d = nc.dram_tensor("Wk", [KROWS, CM_PAD], f32, kind="ExternalInput")
    A2_d = nc.dram_tensor("A2e", [CM_PAD, NSTATE], f32, kind="ExternalInput")
    R_d = nc.dram_tensor("Rt", [9, T * B_local], f32, kind="ExternalInput")
    O_d = nc.dram_tensor("O1", [1, T * B_local], f32, kind="ExternalOutput")

    with tile.TileContext(nc) as tc, ExitStack() as ctx:
        singles = ctx.enter_context(tc.tile_pool(name="singles", bufs=1))
        rpool = ctx.enter_context(tc.tile_pool(name="rstage", bufs=2))
        opool = ctx.enter_context(tc.tile_pool(name="ostage", bufs=2))
        psipool = ctx.enter_context(tc.tile_pool(name="psi", bufs=4))
        rhspool = ctx.enter_context(tc.tile_pool(name="rhs", bufs=6))
        apsum1 = ctx.enter_context(tc.tile_pool(name="apsum1", bufs=2, space="PSUM"))
        apsum2 = ctx.enter_context(tc.tile_pool(name="apsum2", bufs=2, space="PSUM"))
        spsum = ctx.enter_context(tc.tile_pool(name="spsum", bufs=4, space="PSUM"))

        Wsb = singles.tile([KROWS, CM_PAD], f32)
        nc.sync.dma_start(out=Wsb, in_=W_d[:, :])
        A2sb = singles.tile([128, NCHUNK, NSTATE], f32)
        nc.sync.dma_start(out=A2sb, in_=A2_d.rearrange("(c p) n -> p c n", p=128))

        # two independent half-batches per core, software-pipelined so one
        # half's matmuls hide the other half's state-handoff tail
        BH = B_local // 2
        NSTEP = rep * T

        sP = [None, None]       # state psum feeding step t, per half
        rhs = [None, None]      # rhs tile for step t, per half
        rhs_next = [None, None]
        pend_r = [None, None]   # deferred stage-boundary R-copy args
        out_pend = []           # (ost_ap, sp) output rows not yet copied
        Rsts = {}
        Osts = {}

        def col(ti, h):
            return ti * B_local + h * BH

        def prep_rhs(h, t):
            """Allocate + zero + R-rows for half h's step-t rhs (gpsimd)."""
            nsi, nti = divmod(t, STAGE)
            rt = rhspool.tile([KROWS, BH], f32, tag="rhs", name=f"rhs{t}_{h}")
            nc.gpsimd.memset(rt[:, :], 0.0)
            if nsi in Rsts:
                nc.gpsimd.tensor_copy(
                    out=rt[64:KROWS, :],
                    in_=Rsts[nsi][0:9, col(nti, h) : col(nti, h) + BH],
                )
            else:
                pend_r[h] = (rt, nti)
            return rt

        for t in range(NSTEP):
            si, ti = divmod(t, STAGE)
            if ti == 0:
                Rst = rpool.tile([32, STAGE * B_local], f32, tag="rstage")
                rsi = si % (T // STAGE)
                nc.sync.dma_start(
                    out=Rst[0:9, :],
                    in_=R_d[:, rsi * STAGE * B_local : (rsi + 1) * STAGE * B_local],
                )
                Rsts[si] = Rst
                Osts[si] = opool.tile([1, STAGE * B_local], f32, tag="ostage", name=f"ost{si}")
                for h in (0, 1):
                    if pend_r[h] is not None:
                        rt, nti = pend_r[h]
                        nc.gpsimd.tensor_copy(
                            out=rt[64:KROWS, :],
                            in_=Rst[0:9, col(nti, h) : col(nti, h) + BH],
                        )
                        pend_r[h] = None

            for h in (0, 1):
                if rhs[h] is None:
                    rhs[h] = prep_rhs(h, t)
                # handoff tail on DVE alone (measured faster than a
                # parallel DVE/ACT tail: fewer cross-engine waits on mm1);
                # the square reads the fresh SBUF copy, not PSUM
                if sP[h] is not None:
                    nc.vector.tensor_scalar_add(rhs[h][0:NSTATE, :], sP[h], 0.0)
                    nc.vector.tensor_mul(
                        rhs[h][32:48, :], rhs[h][0:NSTATE, :], rhs[h][0:NSTATE, :]
                    )

            # previous step's output rows (DVE, after the tails in FIFO order)
            while out_pend:
                ap, sp = out_pend.pop()
                nc.vector.tensor_scalar_add(ap, sp[0:1, :], 0.0)
            if ti == 0 and si > 0 and (si - 1) in Osts:
                pso = (si - 1) % (T // STAGE)
                nc.sync.dma_start(
                    out=O_d[:, pso * STAGE * B_local : (pso + 1) * STAGE * B_local],
                    in_=Osts[si - 1],
                )
                del Osts[si - 1]

            G1, G2 = (0, 1, 2, 3), (4, 5, 6)
            argp = {}
            sP_new = [None, None]
            for h in (0, 1):  # mm1 for both halves first
                argp[h, 0] = apsum1.tile([128, len(G1) * BH], f32, tag="a1", name=f"a1_{t}_{h}")
                argp[h, 1] = apsum2.tile([128, len(G2) * BH], f32, tag="a2", name=f"a2_{t}_{h}")
                for g, grp in enumerate((G1, G2)):
                    for i, k in enumerate(grp):
                        nc.tensor.matmul(
                            argp[h, g][:, i * BH : (i + 1) * BH],
                            lhsT=Wsb[:, k * 128 : (k + 1) * 128],
                            rhs=rhs[h],
                            start=True,
                            stop=True,
                        )
            psis = {}
            for h in (0, 1):
                for g, grp in enumerate((G1, G2)):
                    psis[h, g] = psipool.tile(
                        [128, len(grp) * BH], f32, tag="psi", name=f"psi{t}_{h}{g}"
                    )
                    nc.scalar.activation(out=psis[h, g], in_=argp[h, g], func=Act.Exp)
            for h in (0, 1):  # mm2 for both halves
                sP_new[h] = spsum.tile([NSTATE, BH], f32, tag="s", name=f"s{t}_{h}")
                for g, grp in enumerate((G1, G2)):
                    for i, k in enumerate(grp):
                        nc.tensor.matmul(
                            sP_new[h],
                            lhsT=A2sb[:, k, :],
                            rhs=psis[h, g][:, i * BH : (i + 1) * BH],
                            start=(k == 0),
                            stop=(k == NCHUNK - 1),
                            skip_group_check=True,
                        )
            for h in (0, 1):
                sP[h] = sP_new[h]
                out_pend.append(
                    (Osts[si][:, col(ti, h) : col(ti, h) + BH], sP[h])
                )
                # prefetch next step's rhs behind the matmuls
                rhs[h] = prep_rhs(h, t + 1) if t + 1 < NSTEP else None

        # final output rows + last stage flush
        while out_pend:
            ap, sp = out_pend.pop()
            nc.vector.tensor_scalar_add(ap, sp[0:1, :], 0.0)
        lsi = NSTEP // STAGE - 1
        lso = lsi % (T // STAGE)
        nc.sync.dma_start(
            out=O_d[:, lso * STAGE * B_local : (lso + 1) * STAGE * B_local],
            in_=Osts[lsi],
        )

    nc.compile()
    return nc


def _host_precompute(x, S, U, A, W1, b1, W2, b2):
    B, T = x.shape
    C, M, N = S.shape
    B_local = B // N_CORES

    # state permutation: put the output component (N-1) at row 0
    perm = np.r_[N - 1, np.arange(N - 1)]

    Sf = S.reshape(C * M, N).astype(np.float32)
    Uf = U.reshape(C * M).astype(np.float32)
    C1 = (Sf * Sf).sum(1) + Uf * Uf
    A2e = np.zeros((CM_PAD, N), np.float32)
    A2e[:CM] = (A.reshape(C * M, N) * np.exp(-C1)[:, None])[:, perm]

    Wk = np.zeros((KROWS, CM_PAD), np.float32)
    Wk[0:N, :CM] = 2.0 * Sf.T[perm]
    Wk[32:48, :CM] = -1.0
    Wk[64, :CM] = 2.0 * Uf
    for c in range(C):
        Wk[65 + c, c * M : (c + 1) * M] = 1.0

    # gate log-softmax, x-only
    h = np.maximum(x[..., None] * W1[0] + b1, 0.0)  # [B,T,16]
    g = h @ W2 + b2  # [B,T,C]
    g = g - g.max(-1, keepdims=True)
    lg = (g - np.log(np.exp(g).sum(-1, keepdims=True))).astype(np.float32)

    x2 = (x * x).astype(np.float32)
    R = np.empty((N_CORES, 9, T, B_local), np.float32)
    for i in range(N_CORES):
        bs = slice(i * B_local, (i + 1) * B_local)
        R[i, 0] = x[bs].T
        R[i, 1:] = (lg[bs] - x2[bs][..., None]).transpose(2, 1, 0)
    R = R.reshape(N_CORES, 9, T * B_local)
    return Wk, A2e, R


def kernel(x, S, U, A, W1, b1, W2, b2):
    x = np.asarray(x, np.float32)
    B, T = x.shape
    assert B % N_CORES == 0 and T % STAGE == 0
    B_local = B // N_CORES

    Wk, A2e, R = _host_precompute(
        np.asarray(x), np.asarray(S), np.asarray(U), np.asarray(A),
        np.asarray(W1), np.asarray(b1), np.asarray(W2), np.asarray(b2),
    )

    key = (B_local, T)
    if key not in _PROGRAM_CACHE:
        _PROGRAM_CACHE[key] = _build_program(B_local, T)
    nc = _PROGRAM_CACHE[key]

    from concourse.bass_utils import run_bass_kernel_spmd

    in_maps = [
        {"Wk": Wk, "A2e": A2e, "Rt": np.ascontiguousarray(R[i])}
        for i in range(N_CORES)
    ]
    res = run_bass_kernel_spmd(nc, in_maps, core_ids=list(range(N_CORES)))
    out = np.empty((B, T), np.float32)
    for i in range(N_CORES):
        O1 = res.results[i]["O1"].reshape(T, B_local)  # [t, b]
        out[i * B_local : (i + 1) * B_local] = O1.T
    return out



# revision 2
# speedup vs baseline: 1.2355x; 1.2355x over previous
"""KAARMA-style multi-cell kernel recurrence on 8 Trainium2 NeuronCores.

Math reformulation (validated vs reference to ~5e-3 rel in bf16):
  per step t, per batch b, for every dictionary atom (c, m) [cm = 800 atoms]:
    phi[b,cm]  = exp(-|s_b - S_cm|^2 - (x_tb - U_cm)^2)
    gate[b,c]  = softmax_c(MLP(x_tb))               (precomputable, x-only)
    s'_b       = sum_cm gate[b,cell(cm)] * phi[b,cm] * A[cm,:]
  Expand the squares and fold the gate into the exponent:
    psi[b,cm]  = exp( 2 s_b.S_cm - |s_b|^2 + 2 x U_cm - x^2 + logsoftmax_c )
    s'_b       = sum_cm psi[b,cm] * (A[cm,:] * exp(-|S_cm|^2 - U_cm^2))
  so one step = matmul([K,B] -> [800,B]) -> exp -> matmul([800,B] -> [16,B]).

All matmul operands are bf16 (PSUM accumulation stays fp32). The x and
log-gate data rows are split into bf16 hi+lo residual pairs on the host so
the exponent keeps ~fp24 effective precision (mm1 cost depends only on the
moving dim, so extra contraction rows are free).

Contraction-row layout (engine SBUF writes must start at partition 0/32/64/96,
so the DVE-written segments are 32-aligned; gaps carry zero weights and are
zeroed once at startup):
  rows  0:16  s            (weights 2*S)
  rows 32:48  s^2          (weights -1)
  rows 64:83  DMA data: x_hi, x_lo, x_hi (again), lgx2_hi[8], lgx2_lo[8]
              (weights 2*U_q, 2*U_q, (2U - 2U_q)_q, one-hot, one-hot)
K = 83; gap rows 16:32, 48:64 are zeroed once and have zero weight columns.

The state for step t+1 is written (bf16) straight into the step-(t+1) column
block of a persistent SBUF "ring" that doubles as the matmul rhs, so there is
no per-step memset/copy of staging tiles.  Ring row 0 (= the permuted output
component of the state) is harvested by a single DMA per 64-step stage.

Sharding: pure data parallel, batch 512 -> 64 per core on 8 cores; each
core's 64 lanes are split into two 32-lane halves that are software-pipelined
against each other to hide the serial-dependence latency of the recurrence.
"""

import numpy as np

N_CORES = 8
CM = 800
CM_PAD = 896
NCHUNK = 7
KROWS = 83
NSTATE = 16
STAGE = 64  # recurrence steps per staging DMA
NRING = 3   # ring buffers (use > 2 so R-prefetch never targets a live buffer)

_PROGRAM_CACHE = {}


def _build_program(B_local, T):
    import concourse.bass as bass
    import concourse.bacc as bacc
    import concourse.tile as tile
    from concourse import mybir
    from contextlib import ExitStack

    f32 = mybir.dt.float32
    bf16 = mybir.dt.bfloat16
    Act = mybir.ActivationFunctionType

    BH = B_local // 2
    SB = STAGE * B_local          # ring columns per stage buffer
    NSTG = T // STAGE

    # Bacc (not Bass): its compile() runs generate_event_semaphores, which
    # splits multi-wait instructions (TRN2 allows 1 wait per instruction)
    nc = bacc.Bacc("TRN2", target_bir_lowering=False, debug=False)
    W_d = nc.dram_tensor("Wk", [KROWS, CM_PAD], bf16, kind="ExternalInput")
    A2_d = nc.dram_tensor("A2e", [CM_PAD, NSTATE], bf16, kind="ExternalInput")
    R_d = nc.dram_tensor("Rt", [19, T * B_local], bf16, kind="ExternalInput")
    O_d = nc.dram_tensor("O1", [1, (T + 1) * B_local], bf16, kind="ExternalOutput")

    with tile.TileContext(nc) as tc, ExitStack() as ctx:
        singles = ctx.enter_context(tc.tile_pool(name="singles", bufs=1))
        psipool = ctx.enter_context(tc.tile_pool(name="psi", bufs=4))
        apsum = ctx.enter_context(tc.tile_pool(name="apsum", bufs=4, space="PSUM"))
        spsum = ctx.enter_context(tc.tile_pool(name="spsum", bufs=4, space="PSUM"))

        Wsb = singles.tile([KROWS, CM_PAD], bf16)
        nc.sync.dma_start(out=Wsb, in_=W_d[:, :])
        A2sb = singles.tile([128, NCHUNK, NSTATE], bf16)
        nc.sync.dma_start(out=A2sb, in_=A2_d.rearrange("(c p) n -> p c n", p=128))

        # persistent state/rhs rings: rows 0:64 hold engine-written state
        # data and static zero gaps, rows 64:83 the per-step DMA data
        rings = [singles.tile([KROWS, SB], bf16, name=f"ring{r}") for r in range(NRING)]
        for r in range(NRING):
            # zero the gap rows (and the initial state block) exactly once
            nc.gpsimd.memset(rings[r][0:64, :], 0.0)
            if r < min(NRING, NSTG):
                nc.sync.dma_start(
                    out=rings[r][64:KROWS, :],
                    in_=R_d[:, r * SB : (r + 1) * SB],
                )

        def slot(t):
            """(ring buffer, column offset) holding step t's rhs block."""
            si, ti = divmod(t, STAGE)
            return rings[si % NRING], ti * B_local

        sP = [None, None]

        for t in range(T):
            si, ti = divmod(t, STAGE)
            if ti == 0 and si > 0:
                # harvest stage si-1's output row (covers steps
                # (si-1)*64-1 .. (si-1)*64+62 -> O_d block offset (si-1)*SB)
                nc.sync.dma_start(
                    out=O_d[:, (si - 1) * SB : si * SB],
                    in_=rings[(si - 1) % NRING][0:1, :],
                )
                if si + 2 < NSTG:
                    nc.sync.dma_start(
                        out=rings[(si + 2) % NRING][64:KROWS, :],
                        in_=R_d[:, (si + 2) * SB : (si + 3) * SB],
                    )

            ring, off = slot(t)
            rhs = [ring[:, off + h * BH : off + (h + 1) * BH] for h in (0, 1)]

            argp = {}
            for h in (0, 1):  # mm1 for both halves first
                argp[h] = apsum.tile([128, NCHUNK * BH], f32, tag="a", name=f"a{t}_{h}")
                for k in range(NCHUNK):
                    nc.tensor.matmul(
                        argp[h][:, k * BH : (k + 1) * BH],
                        lhsT=Wsb[:, k * 128 : (k + 1) * 128],
                        rhs=rhs[h],
                        start=True,
                        stop=True,
                    )
            psis = {}
            for h in (0, 1):
                psis[h] = psipool.tile(
                    [128, NCHUNK * BH], bf16, tag="psi", name=f"psi{t}_{h}"
                )
                nc.scalar.activation(out=psis[h], in_=argp[h], func=Act.Exp)
            for h in (0, 1):  # mm2 for both halves
                sP[h] = spsum.tile([NSTATE, BH], f32, tag="s", name=f"s{t}_{h}")
                for k in range(NCHUNK):
                    nc.tensor.matmul(
                        sP[h],
                        lhsT=A2sb[:, k, :],
                        rhs=psis[h][:, k * BH : (k + 1) * BH],
                        start=(k == 0),
                        stop=(k == NCHUNK - 1),
                        skip_group_check=True,
                    )
            # state handoff into step t+1's rhs block: bf16 copy + square
            nring, noff = slot(t + 1)
            for h in (0, 1):
                dst = nring[:, noff + h * BH : noff + (h + 1) * BH]
                nc.vector.tensor_scalar_add(dst[0:NSTATE, :], sP[h], 0.0)
                nc.vector.tensor_mul(
                    dst[32:48, :], dst[0:NSTATE, :], dst[0:NSTATE, :]
                )

        # final harvests: last stage's buffer + the final state block
        nc.sync.dma_start(
            out=O_d[:, (NSTG - 1) * SB : NSTG * SB],
            in_=rings[(NSTG - 1) % NRING][0:1, :],
        )
        nc.sync.dma_start(
            out=O_d[:, NSTG * SB : NSTG * SB + B_local],
            in_=rings[NSTG % NRING][0:1, 0:B_local],
        )

    nc.compile()
    return nc


def _host_precompute(x, S, U, A, W1, b1, W2, b2):
    import ml_dtypes

    bft = ml_dtypes.bfloat16

    def to_bf(a):
        return np.asarray(a, np.float32).astype(bft)

    B, T = x.shape
    C, M, N = S.shape
    B_local = B // N_CORES

    # state permutation: put the output component (N-1) at row 0
    perm = np.r_[N - 1, np.arange(N - 1)]

    Sf = S.reshape(C * M, N).astype(np.float32)
    Uf = U.reshape(C * M).astype(np.float32)
    C1 = (Sf * Sf).sum(1) + Uf * Uf
    A2e = np.zeros((CM_PAD, N), np.float32)
    A2e[:CM] = (A.reshape(C * M, N) * np.exp(-C1)[:, None])[:, perm]

    Wu = 2.0 * Uf
    Wu_q = to_bf(Wu).astype(np.float32)

    Wk = np.zeros((KROWS, CM_PAD), np.float32)
    Wk[0:N, :CM] = 2.0 * Sf.T[perm]
    Wk[32:48, :CM] = -1.0
    Wk[64, :CM] = Wu_q
    Wk[65, :CM] = Wu_q
    Wk[66, :CM] = Wu - Wu_q  # residual weight row (applied to x_hi)
    for c in range(C):
        Wk[67 + c, c * M : (c + 1) * M] = 1.0  # lgx2_hi one-hot
        Wk[75 + c, c * M : (c + 1) * M] = 1.0  # lgx2_lo one-hot

    # gate log-softmax, x-only
    h = np.maximum(x[..., None] * W1[0] + b1, 0.0)  # [B,T,16]
    g = h @ W2 + b2  # [B,T,C]
    g = g - g.max(-1, keepdims=True)
    lg = (g - np.log(np.exp(g).sum(-1, keepdims=True))).astype(np.float32)

    x2 = (x * x).astype(np.float32)
    lgx2 = lg - x2[..., None]  # [B,T,C]
    x_hi = to_bf(x).astype(np.float32)
    x_lo = x - x_hi
    l_hi = to_bf(lgx2).astype(np.float32)
    l_lo = lgx2 - l_hi

    R = np.empty((N_CORES, 19, T, B_local), np.float32)
    for i in range(N_CORES):
        bs = slice(i * B_local, (i + 1) * B_local)
        R[i, 0] = x_hi[bs].T
        R[i, 1] = x_lo[bs].T
        R[i, 2] = x_hi[bs].T
        R[i, 3:11] = l_hi[bs].transpose(2, 1, 0)
        R[i, 11:19] = l_lo[bs].transpose(2, 1, 0)
    R = R.reshape(N_CORES, 19, T * B_local)
    return to_bf(Wk), to_bf(A2e), to_bf(R)


def kernel(x, S, U, A, W1, b1, W2, b2):
    x = np.asarray(x, np.float32)
    B, T = x.shape
    assert B % N_CORES == 0 and T % STAGE == 0
    B_local = B // N_CORES

    Wk, A2e, R = _host_precompute(
        np.asarray(x), np.asarray(S), np.asarray(U), np.asarray(A),
        np.asarray(W1), np.asarray(b1), np.asarray(W2), np.asarray(b2),
    )

    key = (B_local, T)
    if key not in _PROGRAM_CACHE:
        _PROGRAM_CACHE[key] = _build_program(B_local, T)
    nc = _PROGRAM_CACHE[key]

    from concourse.bass_utils import run_bass_kernel_spmd

    in_maps = [
        {"Wk": Wk, "A2e": A2e, "Rt": np.ascontiguousarray(R[i])}
        for i in range(N_CORES)
    ]
    res = run_bass_kernel_spmd(nc, in_maps, core_ids=list(range(N_CORES)))
    out = np.empty((B, T), np.float32)
    for i in range(N_CORES):
        O1 = res.results[i]["O1"].astype(np.float32).reshape(T + 1, B_local)
        out[i * B_local : (i + 1) * B_local] = O1[1:].T  # drop the t=-1 block
    return out


# revision 5
# speedup vs baseline: 1.2890x; 1.0433x over previous
"""KAARMA-style multi-cell kernel recurrence on 8 Trainium2 NeuronCores.

Math reformulation (validated vs reference to ~5e-3 rel in bf16):
  per step t, per batch b, for every dictionary atom (c, m) [cm = 800 atoms]:
    phi[b,cm]  = exp(-|s_b - S_cm|^2 - (x_tb - U_cm)^2)
    gate[b,c]  = softmax_c(MLP(x_tb))               (precomputable, x-only)
    s'_b       = sum_cm gate[b,cell(cm)] * phi[b,cm] * A[cm,:]
  Expand the squares and fold the gate into the exponent:
    psi[b,cm]  = exp( 2 s_b.S_cm - |s_b|^2 + 2 x U_cm - x^2 + logsoftmax_c )
    s'_b       = sum_cm psi[b,cm] * (A[cm,:] * exp(-|S_cm|^2 - U_cm^2))
  so one step = matmul([K,B] -> [800,B]) -> exp -> matmul([800,B] -> [16,B]).

All matmul operands are bf16 (PSUM accumulation stays fp32). The x and
log-gate data rows are split into bf16 hi+lo residual pairs on the host so
the exponent keeps ~fp24 effective precision (mm1 cost depends only on the
moving dim, so extra contraction rows are free).

Contraction-row layout (engine SBUF writes must start at partition 0/32/64/96,
so the DVE-written segments are 32-aligned; gaps carry zero weights and are
zeroed once at startup):
  rows  0:16  s            (weights 2*S)
  rows 32:48  s^2          (weights -1)
  rows 64:83  DMA data: x_hi, x_lo, x_hi (again), lgx2_hi[8], lgx2_lo[8]
              (weights 2*U_q, 2*U_q, (2U - 2U_q)_q, one-hot, one-hot)
K = 83; gap rows 16:32, 48:64 are zeroed once and have zero weight columns.

The state for step t+1 is written (bf16) straight into the step-(t+1) column
block of a persistent SBUF "ring" that doubles as the matmul rhs, so there is
no per-step memset/copy of staging tiles.  Ring row 0 (= the permuted output
component of the state) is harvested by a single DMA per 64-step stage.

Sharding: pure data parallel, batch 512 -> 64 per core on 8 cores; each
core's 64 lanes are split into two 32-lane halves that are software-pipelined
against each other to hide the serial-dependence latency of the recurrence.
"""

import numpy as np

N_CORES = 8
CM = 800
CM_PAD = 896
NCHUNK = 7
KROWS = 83
NSTATE = 16
STAGE = 64  # recurrence steps per staging DMA
NRING = 3   # ring buffers (use > 2 so R-prefetch never targets a live buffer)

_PROGRAM_CACHE = {}


def _build_program(B_local, T):
    import concourse.bass as bass
    import concourse.bacc as bacc
    import concourse.tile as tile
    from concourse import mybir
    from contextlib import ExitStack

    f32 = mybir.dt.float32
    bf16 = mybir.dt.bfloat16
    Act = mybir.ActivationFunctionType

    NCH = 4                       # independent batch chains (pipelining depth)
    BH = B_local // NCH
    SB = STAGE * B_local          # ring columns per stage buffer
    NSTG = T // STAGE

    # Bacc (not Bass): its compile() runs generate_event_semaphores, which
    # splits multi-wait instructions (TRN2 allows 1 wait per instruction)
    nc = bacc.Bacc("TRN2", target_bir_lowering=False, debug=False)
    W_d = nc.dram_tensor("Wk", [KROWS, CM_PAD], bf16, kind="ExternalInput")
    A2_d = nc.dram_tensor("A2e", [CM_PAD, NSTATE], bf16, kind="ExternalInput")
    R_d = nc.dram_tensor("Rt", [19, T * B_local], bf16, kind="ExternalInput")
    O_d = nc.dram_tensor("O1", [1, (T + 1) * B_local], bf16, kind="ExternalOutput")

    with tile.TileContext(nc) as tc, ExitStack() as ctx:
        singles = ctx.enter_context(tc.tile_pool(name="singles", bufs=1))
        psipool = ctx.enter_context(tc.tile_pool(name="psi", bufs=4))
        apsum = ctx.enter_context(tc.tile_pool(name="apsum", bufs=4, space="PSUM"))
        spsum = ctx.enter_context(tc.tile_pool(name="spsum", bufs=4, space="PSUM"))

        Wsb = singles.tile([KROWS, CM_PAD], bf16)
        nc.sync.dma_start(out=Wsb, in_=W_d[:, :])
        A2sb = singles.tile([128, NCHUNK, NSTATE], bf16)
        nc.sync.dma_start(out=A2sb, in_=A2_d.rearrange("(c p) n -> p c n", p=128))

        # persistent state/rhs rings: rows 0:64 hold engine-written state
        # data and static zero gaps, rows 64:83 the per-step DMA data
        rings = [singles.tile([KROWS, SB], bf16, name=f"ring{r}") for r in range(NRING)]
        for r in range(NRING):
            # zero the gap rows (and the initial state block) exactly once
            nc.gpsimd.memset(rings[r][0:64, :], 0.0)
            if r < min(NRING, NSTG):
                nc.sync.dma_start(
                    out=rings[r][64:KROWS, :],
                    in_=R_d[:, r * SB : (r + 1) * SB],
                )

        def slot(t):
            """(ring buffer, column offset) holding step t's rhs block."""
            si, ti = divmod(t, STAGE)
            return rings[si % NRING], ti * B_local

        sP = [None] * 4

        for t in range(T):
            si, ti = divmod(t, STAGE)
            if ti == 0 and si > 0:
                # harvest stage si-1's output row (covers steps
                # (si-1)*64-1 .. (si-1)*64+62 -> O_d block offset (si-1)*SB)
                nc.sync.dma_start(
                    out=O_d[:, (si - 1) * SB : si * SB],
                    in_=rings[(si - 1) % NRING][0:1, :],
                )
                if si + 2 < NSTG:
                    nc.sync.dma_start(
                        out=rings[(si + 2) % NRING][64:KROWS, :],
                        in_=R_d[:, (si + 2) * SB : (si + 3) * SB],
                    )

            ring, off = slot(t)
            rhs = [ring[:, off + h * BH : off + (h + 1) * BH] for h in range(NCH)]

            argp = {}
            for h in range(NCH):  # mm1 for all chains first
                argp[h] = apsum.tile([128, NCHUNK * BH], f32, tag="a", name=f"a{t}_{h}")
                for k in range(NCHUNK):
                    nc.tensor.matmul(
                        argp[h][:, k * BH : (k + 1) * BH],
                        lhsT=Wsb[:, k * 128 : (k + 1) * 128],
                        rhs=rhs[h],
                        start=True,
                        stop=True,
                    )
            psis = {}
            for h in range(NCH):
                psis[h] = psipool.tile(
                    [128, NCHUNK * BH], bf16, tag="psi", name=f"psi{t}_{h}"
                )
                nc.scalar.activation(out=psis[h], in_=argp[h], func=Act.Exp)
            for h in range(NCH):  # mm2 for all chains
                sP[h] = spsum.tile([NSTATE, BH], f32, tag="s", name=f"s{t}_{h}")
                for k in range(NCHUNK):
                    nc.tensor.matmul(
                        sP[h],
                        lhsT=A2sb[:, k, :],
                        rhs=psis[h][:, k * BH : (k + 1) * BH],
                        start=(k == 0),
                        stop=(k == NCHUNK - 1),
                        skip_group_check=True,
                    )
            # state handoff into step t+1's rhs block: bf16 copy + square
            nring, noff = slot(t + 1)
            for h in range(NCH):
                dst = nring[:, noff + h * BH : noff + (h + 1) * BH]
                nc.vector.tensor_scalar_add(dst[0:NSTATE, :], sP[h], 0.0)
                nc.vector.tensor_mul(
                    dst[32:48, :], dst[0:NSTATE, :], dst[0:NSTATE, :]
                )

        # final harvests: last stage's buffer + the final state block
        nc.sync.dma_start(
            out=O_d[:, (NSTG - 1) * SB : NSTG * SB],
            in_=rings[(NSTG - 1) % NRING][0:1, :],
        )
        nc.sync.dma_start(
            out=O_d[:, NSTG * SB : NSTG * SB + B_local],
            in_=rings[NSTG % NRING][0:1, 0:B_local],
        )

    nc.compile()
    return nc


def _host_precompute(x, S, U, A, W1, b1, W2, b2):
    import ml_dtypes

    bft = ml_dtypes.bfloat16

    def to_bf(a):
        return np.asarray(a, np.float32).astype(bft)

    B, T = x.shape
    C, M, N = S.shape
    B_local = B // N_CORES

    # state permutation: put the output component (N-1) at row 0
    perm = np.r_[N - 1, np.arange(N - 1)]

    Sf = S.reshape(C * M, N).astype(np.float32)
    Uf = U.reshape(C * M).astype(np.float32)
    C1 = (Sf * Sf).sum(1) + Uf * Uf
    A2e = np.zeros((CM_PAD, N), np.float32)
    A2e[:CM] = (A.reshape(C * M, N) * np.exp(-C1)[:, None])[:, perm]

    Wu = 2.0 * Uf
    Wu_q = to_bf(Wu).astype(np.float32)

    Wk = np.zeros((KROWS, CM_PAD), np.float32)
    Wk[0:N, :CM] = 2.0 * Sf.T[perm]
    Wk[32:48, :CM] = -1.0
    Wk[64, :CM] = Wu_q
    Wk[65, :CM] = Wu_q
    Wk[66, :CM] = Wu - Wu_q  # residual weight row (applied to x_hi)
    for c in range(C):
        Wk[67 + c, c * M : (c + 1) * M] = 1.0  # lgx2_hi one-hot
        Wk[75 + c, c * M : (c + 1) * M] = 1.0  # lgx2_lo one-hot

    # gate log-softmax, x-only
    h = np.maximum(x[..., None] * W1[0] + b1, 0.0)  # [B,T,16]
    g = h @ W2 + b2  # [B,T,C]
    g = g - g.max(-1, keepdims=True)
    lg = (g - np.log(np.exp(g).sum(-1, keepdims=True))).astype(np.float32)

    x2 = (x * x).astype(np.float32)
    lgx2 = lg - x2[..., None]  # [B,T,C]
    x_hi = to_bf(x).astype(np.float32)
    x_lo = x - x_hi
    l_hi = to_bf(lgx2).astype(np.float32)
    l_lo = lgx2 - l_hi

    R = np.empty((N_CORES, 19, T, B_local), np.float32)
    for i in range(N_CORES):
        bs = slice(i * B_local, (i + 1) * B_local)
        R[i, 0] = x_hi[bs].T
        R[i, 1] = x_lo[bs].T
        R[i, 2] = x_hi[bs].T
        R[i, 3:11] = l_hi[bs].transpose(2, 1, 0)
        R[i, 11:19] = l_lo[bs].transpose(2, 1, 0)
    R = R.reshape(N_CORES, 19, T * B_local)
    return to_bf(Wk), to_bf(A2e), to_bf(R)


def kernel(x, S, U, A, W1, b1, W2, b2):
    x = np.asarray(x, np.float32)
    B, T = x.shape
    assert B % N_CORES == 0 and T % STAGE == 0
    B_local = B // N_CORES

    Wk, A2e, R = _host_precompute(
        np.asarray(x), np.asarray(S), np.asarray(U), np.asarray(A),
        np.asarray(W1), np.asarray(b1), np.asarray(W2), np.asarray(b2),
    )

    key = (B_local, T)
    if key not in _PROGRAM_CACHE:
        _PROGRAM_CACHE[key] = _build_program(B_local, T)
    nc = _PROGRAM_CACHE[key]

    from concourse.bass_utils import run_bass_kernel_spmd

    in_maps = [
        {"Wk": Wk, "A2e": A2e, "Rt": np.ascontiguousarray(R[i])}
        for i in range(N_CORES)
    ]
    res = run_bass_kernel_spmd(nc, in_maps, core_ids=list(range(N_CORES)))
    out = np.empty((B, T), np.float32)
    for i in range(N_CORES):
        O1 = res.results[i]["O1"].astype(np.float32).reshape(T + 1, B_local)
        out[i * B_local : (i + 1) * B_local] = O1[1:].T  # drop the t=-1 block
    return out


# revision 11
# speedup vs baseline: 1.3259x; 1.0286x over previous
"""KAARMA-style multi-cell kernel recurrence on 8 Trainium2 NeuronCores.

Math reformulation (validated vs reference to ~5e-3 rel in bf16):
  per step t, per batch b, for every dictionary atom (c, m) [cm = 800 atoms]:
    phi[b,cm]  = exp(-|s_b - S_cm|^2 - (x_tb - U_cm)^2)
    gate[b,c]  = softmax_c(MLP(x_tb))               (precomputable, x-only)
    s'_b       = sum_cm gate[b,cell(cm)] * phi[b,cm] * A[cm,:]
  Expand the squares and fold the gate into the exponent:
    psi[b,cm]  = exp( 2 s_b.S_cm - |s_b|^2 + 2 x U_cm - x^2 + logsoftmax_c )
    s'_b       = sum_cm psi[b,cm] * (A[cm,:] * exp(-|S_cm|^2 - U_cm^2))
  so one step = matmul([K,B] -> [800,B]) -> exp -> matmul([800,B] -> [16,B]).

All matmul operands are bf16 (PSUM accumulation stays fp32). The x and
log-gate data rows are split into bf16 hi+lo residual pairs on the host so
the exponent keeps ~fp24 effective precision (mm1 cost depends only on the
moving dim, so extra contraction rows are free).

Contraction-row layout (engine SBUF writes must start at partition 0/32/64/96,
so the DVE-written segments are 32-aligned; gaps carry zero weights and are
zeroed once at startup):
  rows  0:16  s            (weights 2*S)
  rows 32:48  s^2          (weights -1)
  rows 64:83  DMA data: x_hi, x_lo, x_hi (again), lgx2_hi[8], lgx2_lo[8]
              (weights 2*U_q, 2*U_q, (2U - 2U_q)_q, one-hot, one-hot)
K = 83; gap rows 16:32, 48:64 are zeroed once and have zero weight columns.

The state for step t+1 is written (bf16) straight into the step-(t+1) column
block of a persistent SBUF "ring" that doubles as the matmul rhs, so there is
no per-step memset/copy of staging tiles.  Ring row 0 (= the permuted output
component of the state) is harvested by a single DMA per 64-step stage.

Sharding: pure data parallel, batch 512 -> 64 per core on 8 cores; each
core's 64 lanes are split into three independent chains (22/21/21 lanes) that
are software-pipelined against each other so engine work on one chain hides
the serial-dependence (mm1 -> exp -> mm2 -> state handoff) latency of the
others.  The per-step time is the latency of that dependence chain (~1.7 us);
all engines are far from busy-saturated, so chain latency is the binding
constraint (per-instruction init/ack/drain/semaphore costs dominate).
"""

import numpy as np

N_CORES = 8
CM = 800
CM_PAD = 896
NCHUNK = 7
KROWS = 83
NSTATE = 16
STAGE = 64  # recurrence steps per staging DMA
NRING = 3   # ring buffers (use > 2 so R-prefetch never targets a live buffer)

_PROGRAM_CACHE = {}


def _build_program(B_local, T):
    import concourse.bass as bass
    import concourse.bacc as bacc
    import concourse.tile as tile
    from concourse import mybir
    from contextlib import ExitStack

    f32 = mybir.dt.float32
    bf16 = mybir.dt.bfloat16
    Act = mybir.ActivationFunctionType

    NCH = 3                       # independent batch chains (pipelining depth)
    w = [B_local // NCH + (1 if i < B_local % NCH else 0) for i in range(NCH)]
    CH = list(zip([sum(w[:i]) for i in range(NCH)], w))  # (col offset, width)
    SB = STAGE * B_local          # ring columns per stage buffer
    NSTG = T // STAGE

    # Bacc (not Bass): its compile() runs generate_event_semaphores, which
    # splits multi-wait instructions (TRN2 allows 1 wait per instruction)
    nc = bacc.Bacc("TRN2", target_bir_lowering=False, debug=False)
    W_d = nc.dram_tensor("Wk", [KROWS, CM_PAD], bf16, kind="ExternalInput")
    A2_d = nc.dram_tensor("A2e", [CM_PAD, NSTATE], bf16, kind="ExternalInput")
    R_d = nc.dram_tensor("Rt", [19, T * B_local], bf16, kind="ExternalInput")
    O_d = nc.dram_tensor("O1", [1, (T + 1) * B_local], bf16, kind="ExternalOutput")

    with tile.TileContext(nc) as tc, ExitStack() as ctx:
        singles = ctx.enter_context(tc.tile_pool(name="singles", bufs=1))
        psipool = ctx.enter_context(tc.tile_pool(name="psi", bufs=2 * NCH))
        apsum = ctx.enter_context(tc.tile_pool(name="apsum", bufs=4, space="PSUM"))
        spsum = ctx.enter_context(tc.tile_pool(name="spsum", bufs=4, space="PSUM"))

        Wsb = singles.tile([KROWS, CM_PAD], bf16)
        nc.sync.dma_start(out=Wsb, in_=W_d[:, :])
        A2sb = singles.tile([128, NCHUNK, NSTATE], bf16)
        nc.sync.dma_start(out=A2sb, in_=A2_d.rearrange("(c p) n -> p c n", p=128))

        # persistent state/rhs rings: rows 0:64 hold engine-written state
        # data and static zero gaps, rows 64:83 the per-step DMA data
        rings = [singles.tile([KROWS, SB], bf16, name=f"ring{r}") for r in range(NRING)]
        for r in range(NRING):
            # zero the gap rows (and the initial state block) exactly once
            nc.gpsimd.memset(rings[r][0:64, :], 0.0)
            if r < min(NRING, NSTG):
                nc.sync.dma_start(
                    out=rings[r][64:KROWS, :],
                    in_=R_d[:, r * SB : (r + 1) * SB],
                )

        def slot(t):
            """(ring buffer, column offset) holding step t's rhs block."""
            si, ti = divmod(t, STAGE)
            return rings[si % NRING], ti * B_local

        argp = {}
        psis = {}
        sP = {}

        def S0(t, h, u):  # mm1 (+ stage-boundary DMAs on the first unit)
            si, ti = divmod(t, STAGE)
            if h == 0 and ti == 0 and si > 0:
                # harvest stage si-1's output row (covers steps
                # (si-1)*64-1 .. (si-1)*64+62 -> O_d block offset (si-1)*SB)
                nc.sync.dma_start(
                    out=O_d[:, (si - 1) * SB : si * SB],
                    in_=rings[(si - 1) % NRING][0:1, :],
                )
                if si + 2 < NSTG:
                    nc.sync.dma_start(
                        out=rings[(si + 2) % NRING][64:KROWS, :],
                        in_=R_d[:, (si + 2) * SB : (si + 3) * SB],
                    )
            ring, off = slot(t)
            co, BH = CH[h]
            rhs = ring[:, off + co : off + co + BH]
            argp[u] = apsum.tile([128, NCHUNK * BH], f32, tag="a", name=f"a{t}_{h}")
            for k in range(NCHUNK):
                nc.tensor.matmul(
                    argp[u][:, k * BH : (k + 1) * BH],
                    lhsT=Wsb[:, k * 128 : (k + 1) * 128],
                    rhs=rhs,
                    start=True,
                    stop=True,
                )

        def S1(t, h, u):  # exp
            BH = CH[h][1]
            psis[u] = psipool.tile(
                [128, NCHUNK * BH], bf16, tag="psi", name=f"psi{t}_{h}"
            )
            nc.scalar.activation(out=psis[u], in_=argp.pop(u), func=Act.Exp)

        def S2(t, h, u):  # mm2
            BH = CH[h][1]
            sP[u] = spsum.tile([NSTATE, BH], f32, tag="s", name=f"s{t}_{h}")
            psi = psis.pop(u)
            for k in range(NCHUNK):
                nc.tensor.matmul(
                    sP[u],
                    lhsT=A2sb[:, k, :],
                    rhs=psi[:, k * BH : (k + 1) * BH],
                    start=(k == 0),
                    stop=(k == NCHUNK - 1),
                    skip_group_check=True,
                )

        def S3(t, h, u):  # state handoff: bf16 copy + square
            nring, noff = slot(t + 1)
            co, BH = CH[h]
            dst = nring[:, noff + co : noff + co + BH]
            sp = sP.pop(u)
            nc.vector.tensor_scalar_add(dst[0:NSTATE, :], sp, 0.0)
            nc.vector.tensor_mul(
                dst[32:48, :], dst[0:NSTATE, :], dst[0:NSTATE, :]
            )

        # NOTE: emission order per engine IS the engine's FIFO order, and the
        # tile framework only enforces read-after-write for writes emitted
        # before the read — so S3(t, h) must be emitted before S0(t+1, h)
        for t in range(T):
            for h in range(NCH):
                S0(t, h, t * NCH + h)
            for h in range(NCH):
                S1(t, h, t * NCH + h)
            for h in range(NCH):
                S2(t, h, t * NCH + h)
            for h in range(NCH):
                S3(t, h, t * NCH + h)

        # final harvests: last stage's buffer + the final state block
        nc.sync.dma_start(
            out=O_d[:, (NSTG - 1) * SB : NSTG * SB],
            in_=rings[(NSTG - 1) % NRING][0:1, :],
        )
        nc.sync.dma_start(
            out=O_d[:, NSTG * SB : NSTG * SB + B_local],
            in_=rings[NSTG % NRING][0:1, 0:B_local],
        )

    nc.compile()
    return nc


def _host_precompute(x, S, U, A, W1, b1, W2, b2):
    import ml_dtypes

    bft = ml_dtypes.bfloat16

    def to_bf(a):
        return np.asarray(a, np.float32).astype(bft)

    B, T = x.shape
    C, M, N = S.shape
    B_local = B // N_CORES

    # state permutation: put the output component (N-1) at row 0
    perm = np.r_[N - 1, np.arange(N - 1)]

    Sf = S.reshape(C * M, N).astype(np.float32)
    Uf = U.reshape(C * M).astype(np.float32)
    C1 = (Sf * Sf).sum(1) + Uf * Uf
    A2e = np.zeros((CM_PAD, N), np.float32)
    A2e[:CM] = (A.reshape(C * M, N) * np.exp(-C1)[:, None])[:, perm]

    Wu = 2.0 * Uf
    Wu_q = to_bf(Wu).astype(np.float32)

    Wk = np.zeros((KROWS, CM_PAD), np.float32)
    Wk[0:N, :CM] = 2.0 * Sf.T[perm]
    Wk[32:48, :CM] = -1.0
    Wk[64, :CM] = Wu_q
    Wk[65, :CM] = Wu_q
    Wk[66, :CM] = Wu - Wu_q  # residual weight row (applied to x_hi)
    for c in range(C):
        Wk[67 + c, c * M : (c + 1) * M] = 1.0  # lgx2_hi one-hot
        Wk[75 + c, c * M : (c + 1) * M] = 1.0  # lgx2_lo one-hot

    # gate log-softmax, x-only
    h = np.maximum(x[..., None] * W1[0] + b1, 0.0)  # [B,T,16]
    g = h @ W2 + b2  # [B,T,C]
    g = g - g.max(-1, keepdims=True)
    lg = (g - np.log(np.exp(g).sum(-1, keepdims=True))).astype(np.float32)

    x2 = (x * x).astype(np.float32)
    lgx2 = lg - x2[..., None]  # [B,T,C]
    x_hi = to_bf(x).astype(np.float32)
    x_lo = x - x_hi
    l_hi = to_bf(lgx2).astype(np.float32)
    l_lo = lgx2 - l_hi

    R = np.empty((N_CORES, 19, T, B_local), np.float32)
    for i in range(N_CORES):
        bs = slice(i * B_local, (i + 1) * B_local)
        R[i, 0] = x_hi[bs].T
        R[i, 1] = x_lo[bs].T
        R[i, 2] = x_hi[bs].T
        R[i, 3:11] = l_hi[bs].transpose(2, 1, 0)
        R[i, 11:19] = l_lo[bs].transpose(2, 1, 0)
    R = R.reshape(N_CORES, 19, T * B_local)
    return to_bf(Wk), to_bf(A2e), to_bf(R)


def kernel(x, S, U, A, W1, b1, W2, b2):
    x = np.asarray(x, np.float32)
    B, T = x.shape
    assert B % N_CORES == 0 and T % STAGE == 0
    B_local = B // N_CORES

    Wk, A2e, R = _host_precompute(
        np.asarray(x), np.asarray(S), np.asarray(U), np.asarray(A),
        np.asarray(W1), np.asarray(b1), np.asarray(W2), np.asarray(b2),
    )

    key = (B_local, T)
    if key not in _PROGRAM_CACHE:
        _PROGRAM_CACHE[key] = _build_program(B_local, T)
    nc = _PROGRAM_CACHE[key]

    from concourse.bass_utils import run_bass_kernel_spmd

    in_maps = [
        {"Wk": Wk, "A2e": A2e, "Rt": np.ascontiguousarray(R[i])}
        for i in range(N_CORES)
    ]
    res = run_bass_kernel_spmd(nc, in_maps, core_ids=list(range(N_CORES)))
    out = np.empty((B, T), np.float32)
    for i in range(N_CORES):
        O1 = res.results[i]["O1"].astype(np.float32).reshape(T + 1, B_local)
        out[i * B_local : (i + 1) * B_local] = O1[1:].T  # drop the t=-1 block
    return out
